# revision 8
# baseline (speedup 1.0000x reference)
"""Chamfer loss kernel for Trainium2 (8 NeuronCores, SPMD data-parallel over batch).

Problem: pred [8,8192,3], gt [8,8192,3] ->
    scalar = mean_b [ mean_n min_m d(b,n,m) + mean_m min_n d(b,n,m) ]
    d = max(||p-q||^2, 0)

Strategy (one batch element per core):
  - Augmented 5-dim matmul computes the full distance tile directly:
        P~_n = (p0,p1,p2, |p|^2, 1),  Q~_m = (-2q0,-2q1,-2q2, 1, |q|^2)
        dist[n,m] = P~_n . Q~_m
    One K=5 matmul per [128 x 512] output tile (float32r -> full fp32 result
    at 1 cycle/row).
  - Flash-style: PSUM supertiles [128 x 2048], never materialized to HBM.
    DVE keeps a running column-min accumulator [128 x 8192] (tensor_tensor min)
    and running row minima (reduce_min), both straight from PSUM.
  - relu commutes with min, applied after reduction.
  - Per-core output: per-partition row-min sums + colmin sum; host averages.
"""

import os
import sys

import numpy as np

for _p in ("/opt/trn_rl_repo",):
    if os.path.isdir(_p) and _p not in sys.path:
        sys.path.append(_p)

import concourse.bacc as bacc
import concourse.bass as bass
import concourse.mybir as mybir
import concourse.tile as tile
from concourse.bass_utils import run_bass_kernel_spmd
from concourse.masks import make_identity

F32 = mybir.dt.float32
F32R = mybir.dt.float32r
AX = mybir.AxisListType
OP = mybir.AluOpType

BIG = 3.0e38  # "+inf" seed for running minima


def build_chamfer_nc(n: int, m: int, use_f32r: bool = True):
    """Build the per-core chamfer kernel graph.

    Inputs (per core): predT [5, n] f32 (augmented, transposed),
                       gtT   [5, m] f32 (augmented, transposed).
    Output: out [128, 2] f32.
        out[:, 0]  = per-partition sums over n-blocks of relu(row minima)
                     (sum over all of them = sum_n min_m dist)
        out[0, 1]  = sum_m relu(col minima)  (rest of column 1 is zero)
    """
    P = 128
    FREE = 2048  # m supertile (4 PSUM banks)
    MMN = 512  # free dim per matmul (1 PSUM bank, fp32)
    assert n % P == 0 and m % FREE == 0
    NB = n // P
    MS = m // FREE
    NMM = FREE // MMN

    nc = bacc.Bacc("TRN2", target_bir_lowering=False, debug=False)
    mm_dt = F32R if use_f32r else F32
    predT_d = nc.dram_tensor("predT", [5, n], mm_dt, kind="ExternalInput")
    gtT_d = nc.dram_tensor("gtT", [5, m], mm_dt, kind="ExternalInput")
    out_d = nc.dram_tensor("out", [P, 2], F32, kind="ExternalOutput")

    with tile.TileContext(nc) as tc:
        with (
            tc.tile_pool(name="const", bufs=1) as cpool,
            tc.tile_pool(name="psum", bufs=2, space=bass.MemorySpace.PSUM) as ppool,
            tc.tile_pool(name="work", bufs=2) as wpool,
        ):
            predT = cpool.tile([5, n], mm_dt)
            gtT = cpool.tile([5, m], mm_dt)
            nc.sync.dma_start(predT[:], predT_d[:])
            nc.sync.dma_start(gtT[:], gtT_d[:])

            colacc = cpool.tile([P, m], F32)
            nc.gpsimd.memset(colacc[:], BIG)
            rowmins = cpool.tile([P, NB], F32)

            for i in range(NB):
                lhsT = predT[:, i * P : (i + 1) * P]
                rowpart = wpool.tile([P, MS], F32, tag="rowpart")
                for J in range(MS):
                    acc = ppool.tile([P, FREE], F32, tag="acc")
                    for j in range(NMM):
                        rhs = gtT[:, J * FREE + j * MMN : J * FREE + (j + 1) * MMN]
                        nc.tensor.matmul(
                            acc[:, j * MMN : (j + 1) * MMN],
                            lhsT,
                            rhs,
                            start=True,
                            stop=True,
                        )
                    # running row minima (partial per supertile)
                    nc.vector.tensor_reduce(
                        rowpart[:, J : J + 1], acc[:], axis=AX.X, op=OP.min
                    )
                    # running column minima
                    cslice = colacc[:, J * FREE : (J + 1) * FREE]
                    nc.vector.tensor_tensor(cslice, acc[:], cslice, op=OP.min)
                nc.vector.tensor_reduce(
                    rowmins[:, i : i + 1], rowpart[:], axis=AX.X, op=OP.min
                )

            # ---- finalize ----
            # rows: relu then sum -> [P, 1]
            rowrelu = cpool.tile([P, NB], F32)
            rowsum = cpool.tile([P, 1], F32)
            nc.vector.tensor_scalar_max(rowrelu[:], rowmins[:], 0.0)
            nc.vector.tensor_reduce(rowsum[:], rowrelu[:], axis=AX.X, op=OP.add)

            # cols: DVE cannot reduce across partitions. Transpose colacc in
            # 128x128 chunks on the PE, then reduce over the free axis.
            # colminT[mm, k] = min over n-partitions of colacc chunk k.
            ident = cpool.tile([P, P], F32)
            make_identity(nc, ident[:])
            NCH = m // P
            colminT = cpool.tile([P, NCH], F32)
            for k in range(NCH):
                tp = ppool.tile([P, FREE], F32, tag="acc")
                nc.tensor.transpose(
                    tp[:, 0:P], colacc[:, k * P : (k + 1) * P], ident[:]
                )
                nc.vector.tensor_reduce(
                    colminT[:, k : k + 1], tp[:, 0:P], axis=AX.X, op=OP.min
                )
            colrelu = cpool.tile([P, NCH], F32)
            colsum = cpool.tile([P, 1], F32)
            nc.vector.tensor_scalar_max(colrelu[:], colminT[:], 0.0)
            nc.vector.tensor_reduce(colsum[:], colrelu[:], axis=AX.X, op=OP.add)

            out_sb = cpool.tile([P, 2], F32)
            nc.vector.tensor_copy(out_sb[:, 0:1], rowsum[:])
            nc.vector.tensor_copy(out_sb[:, 1:2], colsum[:])
            nc.sync.dma_start(out_d[:], out_sb[:])

    nc.compile()
    return nc


def _augment(pred: np.ndarray, gt: np.ndarray):
    """pred [n,3], gt [m,3] f32 -> predT [5,n], gtT [5,m] f32."""
    n, m = pred.shape[0], gt.shape[0]
    predT = np.empty((5, n), np.float32)
    predT[0:3] = pred.T
    predT[3] = np.sum(pred.astype(np.float64) ** 2, axis=-1).astype(np.float32)
    predT[4] = 1.0
    gtT = np.empty((5, m), np.float32)
    gtT[0:3] = -2.0 * gt.T
    gtT[3] = 1.0
    gtT[4] = np.sum(gt.astype(np.float64) ** 2, axis=-1).astype(np.float32)
    return predT, gtT


_NC_CACHE = {}


def _get_nc(n, m, use_f32r=True):
    key = (n, m, use_f32r)
    if key not in _NC_CACHE:
        _NC_CACHE[key] = build_chamfer_nc(n, m, use_f32r)
    return _NC_CACHE[key]


def run_chamfer(pred: np.ndarray, gt: np.ndarray, use_f32r: bool = True, **kw):
    """pred [B,N,3], gt [B,M,3] -> (scalar, BassKernelResults)."""
    B, N, _ = pred.shape
    M = gt.shape[1]
    assert B <= 8
    nc = _get_nc(N, M, use_f32r)
    in_maps = []
    for b in range(B):
        predT, gtT = _augment(
            np.ascontiguousarray(pred[b], np.float32),
            np.ascontiguousarray(gt[b], np.float32),
        )
        in_maps.append({"predT": predT, "gtT": gtT})
    res = run_bass_kernel_spmd(nc, in_maps, core_ids=list(range(B)), **kw)
    vals = []
    for r in res.results:
        o = r["out"]
        p2q = float(o[:, 0].sum()) / N
        q2p = float(o[:, 1].sum()) / M
        vals.append(p2q + q2p)
    return np.float32(np.mean(vals)), res


def kernel(pred: np.ndarray, gt: np.ndarray) -> np.ndarray:
    val, _ = run_chamfer(np.asarray(pred), np.asarray(gt))
    return np.array(val, dtype=np.float32)
